# revision 9
# baseline (speedup 1.0000x reference)
"""Trainium2 Bass kernel for batched dense attention (v5: pair AllGather).

Reference (per batch b):
    q = query @ Wq + bq ; k = key @ Wk + bk ; v = value @ Wv + bv
    out = softmax(BETA * q k^T) v

Sharding: 8 cores = (batch b, seq half h). Core (b,h) computes out rows
[b, h*1024:(h+1)*1024, :].

v5 removes the K-side duplication of v3: each core projects only its OWN
half of the keys (kTr, 65k cycles instead of 131k) and additionally
pre-projects its own half of V' = value @ Wv + bv (65k cycles); the halves
are exchanged within each core pair via DRAM-bounce AllGather
(replica_groups [[0,1],[2,3],[4,5],[6,7]]) overlapped under ~50us of
independent PE work (V'o + q-projection for the kTr gather, phase A for
the V' gather). With V' resident, the attention epilogue is just
out = (P V') * (1/rowsum)  -- v3's phase C (65k cycles) and its PSUM
transposes/copies disappear, and bv rides inside V' exactly
(P(V'+bv)*rrec = PV'*rrec + bv since P@1*rrec == 1).

The program is h-agnostic (SPMD-safe): local results go to a staging tile
-> AllGather input bounce; BOTH halves of kTr/V' are read back from the
gathered output (rank r occupies rows r*128..(r+1)*128), so no core-id
branching is needed.

Per-core PE budget @2.4GHz: kproj-own 65k + V'own 65k + qproj 65k +
scores 131k + PV' 131k ~= 458k cycles ~= 191us (vs v3's 523k ~= 218us).
"""
import ml_dtypes
import numpy as np

import concourse.bass as bass
import concourse.bacc as bacc
import concourse.tile as tile
from concourse import mybir
from concourse.bass_utils import run_bass_kernel_spmd

B, S, D = 4, 2048, 1024
KD = 1024
VD = 1024
BETA = 1.0 / float(np.sqrt(D))
N_CORES = 8
QS = S // 2          # per-core query rows (1024)
KH = S // 2          # own key half (1024)

F32 = mybir.dt.float32
BF16 = mybir.dt.bfloat16

C_D = D // 128       # 8 contraction chunks over D
G_KD = KD // 128     # 8 kd chunks
KT = S // 128        # 16 key tiles (full)
QBLK = 512
NQB = QS // QBLK     # 2 q blocks
NQS = QBLK // 128    # 4 q slices per block

REPLICA_GROUPS = [[2 * i, 2 * i + 1] for i in range(4)]


def build_kernel():
    nc = bacc.Bacc("TRN2", target_bir_lowering=False, debug=False,
                   num_devices=N_CORES)

    # host-prearranged [128, ...] SBUF layouts: inputs as (block, chunk, 512)
    # columns, weights as (chunk, cols) -- every DMA is a contiguous 2D copy
    qT = nc.dram_tensor("qT", [128, 2 * C_D * 512], BF16,
                        kind="ExternalInput").ap()
    kTo = nc.dram_tensor("kTo", [128, 2 * C_D * 512], BF16,
                         kind="ExternalInput").ap()
    vTo = nc.dram_tensor("vTo", [128, 2 * C_D * 512], BF16,
                         kind="ExternalInput").ap()
    Wq = nc.dram_tensor("Wq", [128, C_D * KD], BF16,
                        kind="ExternalInput").ap()
    Wk = nc.dram_tensor("Wk", [128, C_D * KD], BF16,
                        kind="ExternalInput").ap()
    Wv16 = nc.dram_tensor("Wv16", [128, C_D * VD], BF16,
                          kind="ExternalInput").ap()
    bqk = nc.dram_tensor("bqk", [128, 16], F32, kind="ExternalInput").ap()
    bv = nc.dram_tensor("bv", [VD], F32, kind="ExternalInput").ap()
    out = nc.dram_tensor("out", [QS, VD], F32, kind="ExternalOutput").ap()

    with tile.TileContext(nc) as tc:
        _body(tc, qT, kTo, vTo, Wq, Wk, Wv16, bqk, bv, out)
    nc.compile()
    return nc


def _chunked(dram_ap, rows0, nchunk, cols):
    sl = dram_ap[rows0:rows0 + nchunk * 128, 0:cols]
    return sl.rearrange("(c p) x -> p c x", c=nchunk)


def _body(tc, qT, kTo, vTo, Wq, Wk, Wv16, bqk, bv, out):
    nc = tc.nc
    Exp = mybir.ActivationFunctionType.Exp
    mult = mybir.AluOpType.mult
    add = mybir.AluOpType.add

    # ---- persistent constants ------------------------------------------
    const_pool = tc.alloc_tile_pool(name="const", bufs=1)
    constf = const_pool.tile([128, 2210], F32, name="constf")
    bqT = constf[:, 0:8]
    bkT = constf[:, 8:16]
    bvb = constf[:, 16:16 + VD]
    ones_f = constf[:, 1040:1042]
    rrec_all = constf[:, 1042:1058]
    onesrow_f = constf[0:1, 1058:1058 + 128]
    bv_stage = constf[0:1, 1186:1186 + VD]
    onesb = const_pool.tile([128, 2], BF16, name="onesb")

    nc.scalar.dma_start(out=constf[:, 0:16], in_=bqk[:, :])
    nc.scalar.dma_start(out=bv_stage, in_=bv[:])
    nc.vector.memset(ones_f, 1.0)
    nc.vector.memset(onesrow_f, 1.0)
    nc.vector.tensor_copy(onesb[:], ones_f)

    # ---- persistent activations ----------------------------------------
    big_pool = tc.alloc_tile_pool(name="big", bufs=1)
    qTr = big_pool.tile([128, G_KD * QS], BF16, name="qTr")       # 16KB/p
    # kTr2 layout: [128, h(2) x g(8) x 1024]  (rank-major halves)
    kTr2 = big_pool.tile([128, 2 * G_KD * KH], BF16, name="kTr2")  # 32KB/p
    Wv_sb = big_pool.tile([128, C_D * VD], BF16, name="Wv_sb")    # 16KB/p
    pT = big_pool.tile([128, KT * QBLK], BF16, name="pT")         # 16KB/p
    # V'sb layout: [128, kt(16) x 1024] (kt tile = k rows, cols = vd)
    Vp = big_pool.tile([128, KT * VD], BF16, name="Vp")           # 32KB/p
    ost_all = big_pool.tile([128, 2 * 1024], F32, name="ost_all")  # 8KB/p
    ostage = [ost_all[:, i * 1024:(i + 1) * 1024] for i in range(2)]

    # DRAM bounce buffers for the pair AllGathers (one pair per half so
    # each gather dispatches as soon as its half of the projection is done)
    dram_pool = tc.alloc_tile_pool(name="dramp", bufs=1, space="DRAM")
    kAG_in = [dram_pool.tile([128, 4 * 1024], BF16, name=f"kAG_in{i}")
              for i in range(2)]
    kAG_out = [dram_pool.tile([256, 4 * 1024], BF16, name=f"kAG_out{i}")
               for i in range(2)]
    vAG_in = [dram_pool.tile([128, 4 * VD], BF16, name=f"vAG_in{i}")
              for i in range(2)]
    vAG_out = [dram_pool.tile([256, 4 * VD], BF16, name=f"vAG_out{i}")
               for i in range(2)]

    # ---- projection-phase transients -----------------------------------
    proj_pool = tc.alloc_tile_pool(name="proj", bufs=1)
    Wk_sb = proj_pool.tile([128, C_D * KD], BF16, name="Wk_sb")   # 16KB/p
    Wq_sb = proj_pool.tile([128, C_D * KD], BF16, name="Wq_sb")   # 16KB/p
    stg = proj_pool.tile([128, G_KD * 1024], BF16, name="stg")    # 16KB/p

    BLKC = C_D * 512

    def xin_tile(engine, src_ap, name):
        t = proj_pool.tile([128, BLKC], BF16, name=name, tag="xin", bufs=3)
        engine.dma_start(out=t[:], in_=src_ap)
        return t

    # scalar queue: kTo blocks (kproj gate), then vTo blocks, then Wv
    kin = [xin_tile(nc.scalar, kTo[:, b * BLKC:(b + 1) * BLKC], f"kin{b}")
           for b in range(2)]
    vin = [xin_tile(nc.scalar, vTo[:, b * BLKC:(b + 1) * BLKC], f"vin{b}")
           for b in range(2)]
    # Wv needed by V'o (~45us in); scalar queue, after the kproj gates.
    # gpsimd stays reserved for the bounce writes + collectives.
    nc.scalar.dma_start(out=Wv_sb[:], in_=Wv16[:, :])
    # sync queue: Wk first (kproj gate), then Wq/qT (consumed later, load
    # during kproj). Gather read-backs ride the scalar queue, which drains
    # its loads by ~25us -- keeping them off sync avoids blocking Wq/qT
    # behind the collectives (in-order FIFO per engine).
    nc.sync.dma_start(out=Wk_sb[:], in_=Wk[:, :])
    nc.sync.dma_start(out=Wq_sb[:], in_=Wq[:, :])
    qin = [xin_tile(nc.sync, qT[:, b * BLKC:(b + 1) * BLKC], f"qin{b}")
           for b in range(2)]

    psPro = tc.alloc_tile_pool(name="psPro", bufs=1, space="PSUM")

    # ---- k projection (own half): stg[g,1024] = (Wk^T kTo) + bk --------
    # g-half ordered so each half of stg completes early and its AllGather
    # dispatches while the other half is still computing.
    HG = G_KD // 2
    for gh in range(2):
        for blk in range(2):
            pps = [psPro.tile([128, 512], F32, name=f"kp{gh}_{blk}_{j}",
                              tag="pp", bufs=8) for j in range(HG)]
            for c in range(C_D):
                for j in range(HG):
                    g = gh * HG + j
                    nc.tensor.matmul(
                        pps[j][:],
                        Wk_sb[:, c * KD + g * 128:c * KD + (g + 1) * 128],
                        kin[blk][:, c * 512:(c + 1) * 512],
                        start=(c == 0), stop=(c == C_D - 1))
            for j in range(HG):
                g = gh * HG + j
                nc.vector.tensor_scalar(
                    out=stg[:, g * 1024 + blk * 512:
                            g * 1024 + (blk + 1) * 512],
                    in0=pps[j][:], scalar1=bkT[:, g:g + 1], scalar2=None,
                    op0=add)
        # ship this g-half, gather it, read both ranks' halves back
        h0 = gh * HG * 1024
        nc.gpsimd.dma_start(out=kAG_in[gh][:], in_=stg[:, h0:h0 + HG * 1024])
        nc.gpsimd.collective_compute(
            "AllGather", mybir.AluOpType.bypass,
            replica_groups=REPLICA_GROUPS,
            ins=[kAG_in[gh][:]], outs=[kAG_out[gh][:]])
        for r in range(2):
            nc.scalar.dma_start(
                out=kTr2[:, r * G_KD * KH + h0:r * G_KD * KH + h0 + HG * 1024],
                in_=kAG_out[gh][r * 128:(r + 1) * 128, :])

    # bv broadcast to all partitions via K=1 fp32 matmul
    for n in range(VD // 512):
        bc_ps = psPro.tile([128, 512], F32, name="bc_ps", tag="pp", bufs=8)
        nc.tensor.matmul(bc_ps[:], onesrow_f,
                         bv_stage[:, n * 512:(n + 1) * 512],
                         start=True, stop=True)
        nc.vector.tensor_copy(bvb[:, n * 512:(n + 1) * 512], bc_ps[:])

    # ---- V' (own half): stg[rt,1024] = value_own @ Wv + bv --------------
    # lhsT = vTo chunks [d, k-row slice], rhs = Wv chunks [d, vd cols]
    for blk in range(2):
        pps = [psPro.tile([128, 512], F32, name=f"vp{blk}_{i}", tag="pp",
                          bufs=8) for i in range(G_KD)]
        for c in range(C_D):
            for i in range(G_KD):
                rt, col = divmod(i, 2)
                nc.tensor.matmul(
                    pps[i][:],
                    vin[blk][:, c * 512 + rt * 128:c * 512 + (rt + 1) * 128],
                    Wv_sb[:, c * VD + col * 512:c * VD + (col + 1) * 512],
                    start=(c == 0), stop=(c == C_D - 1))
        for i in range(G_KD):
            rt, col = divmod(i, 2)
            krow = blk * 4 + rt
            nc.vector.tensor_tensor(
                out=stg[:, krow * 1024 + col * 512:krow * 1024 + (col + 1) * 512],
                in0=pps[i][:], in1=bvb[:, col * 512:(col + 1) * 512], op=add)
        # ship this krow-half, gather, read both ranks' quarters back
        b0 = blk * 4 * 1024
        nc.gpsimd.dma_start(out=vAG_in[blk][:], in_=stg[:, b0:b0 + 4 * 1024])
        nc.gpsimd.collective_compute(
            "AllGather", mybir.AluOpType.bypass,
            replica_groups=REPLICA_GROUPS,
            ins=[vAG_in[blk][:]], outs=[vAG_out[blk][:]])
        for r in range(2):
            nc.scalar.dma_start(
                out=Vp[:, r * 8 * VD + b0:r * 8 * VD + b0 + 4 * VD],
                in_=vAG_out[blk][r * 128:(r + 1) * 128, :])

    # ---- q projection: qTr[kd, q] = (Wq^T qT) + bq ----------------------
    for blk in range(NQB):
        pps = [psPro.tile([128, 512], F32, name=f"qp{blk}_{g}", tag="pp",
                          bufs=8) for g in range(G_KD)]
        for c in range(C_D):
            for g in range(G_KD):
                nc.tensor.matmul(
                    pps[g][:],
                    Wq_sb[:, c * KD + g * 128:c * KD + (g + 1) * 128],
                    qin[blk][:, c * 512:(c + 1) * 512],
                    start=(c == 0), stop=(c == C_D - 1))
        for g in range(G_KD):
            nc.vector.tensor_scalar(
                out=qTr[:, g * QS + blk * 512:g * QS + (blk + 1) * 512],
                in0=pps[g][:], scalar1=bqT[:, g:g + 1], scalar2=None, op0=add)

    psPro.release()
    proj_pool.release()

    # ===== main attention loop ==========================================
    # PSUM: sT(2) + rs(1) + acc(4) = 7 banks.
    psM = tc.alloc_tile_pool(name="psM", bufs=1, space="PSUM")
    rs_ps = psM.tile([128, 2 * NQS], F32, name="rs_ps", tag="rs")

    def kslice(g, kt):
        h, kk = divmod(kt, 8)
        base = h * G_KD * KH + g * KH + kk * 128
        return kTr2[:, base:base + 128]

    for qb in range(NQB):
        q0 = qb * QBLK
        # ---- phase A: sT = kTr^T qTr -> exp -> pT ; rowsums on PE ------
        for kt in range(KT):
            sT = psM.tile([128, QBLK], F32, name=f"sT{qb}_{kt}", tag="sT",
                          bufs=2)
            for g in range(G_KD):
                nc.tensor.matmul(
                    sT[:], kslice(g, kt),
                    qTr[:, g * QS + q0:g * QS + q0 + QBLK],
                    start=(g == 0), stop=(g == G_KD - 1))
            nc.scalar.activation(pT[:, kt * QBLK:(kt + 1) * QBLK], sT[:],
                                 Exp, scale=float(BETA))
            for qs in range(NQS):
                nc.tensor.matmul(
                    rs_ps[:, 2 * qs:2 * qs + 2],
                    pT[:, kt * QBLK + qs * 128:kt * QBLK + (qs + 1) * 128],
                    onesb[:],
                    start=(kt == 0 and qs == 0),
                    stop=(kt == KT - 1 and qs == NQS - 1),
                    skip_group_check=True)
        rrec = rrec_all[:, qb * 2 * NQS:(qb + 1) * 2 * NQS]
        nc.vector.reciprocal(rrec, rs_ps[:])

        # ---- phase B': out = (P V') * rrec  (V' already includes bv) ----
        for qs in range(NQS):
            accs = [psM.tile([128, 512], F32, name=f"ob{qb}_{qs}_{p}",
                             tag="acc", bufs=4) for p in range(2)]
            for kt in range(KT):
                for p in range(2):
                    nc.tensor.matmul(
                        accs[p][:],
                        pT[:, kt * QBLK + qs * 128:kt * QBLK + (qs + 1) * 128],
                        Vp[:, kt * VD + p * 512:kt * VD + (p + 1) * 512],
                        start=(kt == 0), stop=(kt == KT - 1))
            ost = ostage[qs % 2]
            for p in range(2):
                nc.vector.tensor_scalar(
                    out=ost[:, p * 512:(p + 1) * 512], in0=accs[p][:],
                    scalar1=rrec[:, 2 * qs:2 * qs + 1], scalar2=None, op0=mult)
            nc.sync.dma_start(
                out=out[q0 + qs * 128:q0 + (qs + 1) * 128, :], in_=ost[:])

    psM.release()
    dram_pool.release()
    big_pool.release()
    const_pool.release()


_NC_CACHE = {}


def _get_nc():
    if "nc" not in _NC_CACHE:
        _NC_CACHE["nc"] = build_kernel()
    return _NC_CACHE["nc"]


def kernel(query, key, value, Wq, bq, Wk, bk, Wv, bv):
    query = np.asarray(query, dtype=np.float32)
    key = np.asarray(key, dtype=np.float32)
    value = np.asarray(value, dtype=np.float32)
    Wq = np.asarray(Wq, dtype=np.float32)
    Wk = np.asarray(Wk, dtype=np.float32)
    Wv = np.asarray(Wv, dtype=np.float32)
    bq = np.asarray(bq, dtype=np.float32)
    bk = np.asarray(bk, dtype=np.float32)
    bv = np.ascontiguousarray(np.asarray(bv, dtype=np.float32))

    nc = _get_nc()
    in_maps = make_in_maps(query, key, value, Wq, bq, Wk, bk, Wv, bv)
    res = run_bass_kernel_spmd(nc, in_maps, list(range(N_CORES)))
    outp = np.empty((B, S, VD), dtype=np.float32)
    for core in range(N_CORES):
        b, h = divmod(core, 2)
        outp[b, h * QS:(h + 1) * QS, :] = res.results[core]["out"]
    return outp


def _arrange_w(W):
    """[D, N] f32 -> bf16 [128, C_D*N], columns (chunk, col)."""
    Dn, N = W.shape
    return np.ascontiguousarray(
        W.astype(ml_dtypes.bfloat16).reshape(C_D, 128, N)
        .transpose(1, 0, 2).reshape(128, C_D * N))


def _arrange_xt(Xt):
    """[D, 1024] f32 (transposed input) -> bf16 [128, 2*C_D*512],
    columns (block, chunk, col)."""
    return np.ascontiguousarray(
        Xt.astype(ml_dtypes.bfloat16).reshape(C_D, 128, 2, 512)
        .transpose(1, 2, 0, 3).reshape(128, 2 * C_D * 512))


def make_in_maps(query, key, value, Wq, bq, Wk, bk, Wv, bv):
    Wq16 = _arrange_w(Wq)
    Wk16 = _arrange_w(Wk)
    Wv16 = _arrange_w(Wv)
    bqk = np.ascontiguousarray(
        np.concatenate([bq.reshape(8, 128).T, bk.reshape(8, 128).T], axis=1)
        .astype(np.float32))
    in_maps = []
    for core in range(N_CORES):
        b, h = divmod(core, 2)
        sl = slice(h * KH, (h + 1) * KH)
        in_maps.append({
            "qT": _arrange_xt(query[b, h * QS:(h + 1) * QS, :].T),
            "kTo": _arrange_xt(key[b, sl, :].T),
            "vTo": _arrange_xt(value[b, sl, :].T),
            "Wq": Wq16, "Wk": Wk16, "Wv16": Wv16,
            "bqk": bqk, "bv": bv,
        })
    return in_maps


# revision 10
# speedup vs baseline: 1.0945x; 1.0945x over previous
"""Trainium2 Bass kernel for batched dense attention (v5: pair AllGather).

Reference (per batch b):
    q = query @ Wq + bq ; k = key @ Wk + bk ; v = value @ Wv + bv
    out = softmax(BETA * q k^T) v

Sharding: 8 cores = (batch b, seq half h). Core (b,h) computes out rows
[b, h*1024:(h+1)*1024, :].

v5 removes the K-side duplication of v3: each core projects only its OWN
half of the keys (kTr, 65k cycles instead of 131k) and additionally
pre-projects its own half of V' = value @ Wv + bv (65k cycles); the halves
are exchanged within each core pair via DRAM-bounce AllGather
(replica_groups [[0,1],[2,3],[4,5],[6,7]]) overlapped under ~50us of
independent PE work (V'o + q-projection for the kTr gather, phase A for
the V' gather). With V' resident, the attention epilogue is just
out = (P V') * (1/rowsum)  -- v3's phase C (65k cycles) and its PSUM
transposes/copies disappear, and bv rides inside V' exactly
(P(V'+bv)*rrec = PV'*rrec + bv since P@1*rrec == 1).

The program is h-agnostic (SPMD-safe): local results go to a staging tile
-> AllGather input bounce; BOTH halves of kTr/V' are read back from the
gathered output (rank r occupies rows r*128..(r+1)*128), so no core-id
branching is needed.

Per-core PE budget @2.4GHz: kproj-own 65k + V'own 65k + qproj 65k +
scores 131k + PV' 131k ~= 458k cycles ~= 191us (vs v3's 523k ~= 218us).
"""
import ml_dtypes
import numpy as np

import concourse.bass as bass
import concourse.bacc as bacc
import concourse.tile as tile
from concourse import mybir
from concourse.bass_utils import run_bass_kernel_spmd

B, S, D = 4, 2048, 1024
KD = 1024
VD = 1024
BETA = 1.0 / float(np.sqrt(D))
N_CORES = 8
QS = S // 2          # per-core query rows (1024)
KH = S // 2          # own key half (1024)

F32 = mybir.dt.float32
BF16 = mybir.dt.bfloat16

C_D = D // 128       # 8 contraction chunks over D
G_KD = KD // 128     # 8 kd chunks
KT = S // 128        # 16 key tiles (full)
QBLK = 512
NQB = QS // QBLK     # 2 q blocks
NQS = QBLK // 128    # 4 q slices per block

REPLICA_GROUPS = [[2 * i, 2 * i + 1] for i in range(4)]


def build_kernel():
    nc = bacc.Bacc("TRN2", target_bir_lowering=False, debug=False,
                   num_devices=N_CORES)

    qT = nc.dram_tensor("qT", [D, QS], BF16, kind="ExternalInput").ap()
    kTo = nc.dram_tensor("kTo", [D, KH], BF16, kind="ExternalInput").ap()
    vTo = nc.dram_tensor("vTo", [D, KH], BF16, kind="ExternalInput").ap()
    Wq = nc.dram_tensor("Wq", [D, KD], BF16, kind="ExternalInput").ap()
    Wk = nc.dram_tensor("Wk", [D, KD], BF16, kind="ExternalInput").ap()
    Wv16 = nc.dram_tensor("Wv16", [VD, VD], BF16, kind="ExternalInput").ap()
    bqk = nc.dram_tensor("bqk", [128, 16], F32, kind="ExternalInput").ap()
    bv = nc.dram_tensor("bv", [VD], F32, kind="ExternalInput").ap()
    out = nc.dram_tensor("out", [QS, VD], F32, kind="ExternalOutput").ap()

    with tile.TileContext(nc) as tc:
        _body(tc, qT, kTo, vTo, Wq, Wk, Wv16, bqk, bv, out)
    nc.compile()
    return nc


def _chunked(dram_ap, rows0, nchunk, cols):
    sl = dram_ap[rows0:rows0 + nchunk * 128, 0:cols]
    return sl.rearrange("(c p) x -> p c x", c=nchunk)


def _body(tc, qT, kTo, vTo, Wq, Wk, Wv16, bqk, bv, out):
    nc = tc.nc
    Exp = mybir.ActivationFunctionType.Exp
    mult = mybir.AluOpType.mult
    add = mybir.AluOpType.add

    # ---- persistent constants ------------------------------------------
    const_pool = tc.alloc_tile_pool(name="const", bufs=1)
    constf = const_pool.tile([128, 2210], F32, name="constf")
    bqT = constf[:, 0:8]
    bkT = constf[:, 8:16]
    bvb = constf[:, 16:16 + VD]
    ones_f = constf[:, 1040:1042]
    rrec_all = constf[:, 1042:1058]
    onesrow_f = constf[0:1, 1058:1058 + 128]
    bv_stage = constf[0:1, 1186:1186 + VD]
    onesb = const_pool.tile([128, 2], BF16, name="onesb")

    nc.scalar.dma_start(out=constf[:, 0:16], in_=bqk[:, :])
    nc.scalar.dma_start(out=bv_stage, in_=bv[:])
    nc.vector.memset(ones_f, 1.0)
    nc.vector.memset(onesrow_f, 1.0)
    nc.vector.tensor_copy(onesb[:], ones_f)

    # ---- persistent activations ----------------------------------------
    big_pool = tc.alloc_tile_pool(name="big", bufs=1)
    qTr = big_pool.tile([128, G_KD * QS], BF16, name="qTr")       # 16KB/p
    # kTr2 layout: [128, h(2) x g(8) x 1024]  (rank-major halves)
    kTr2 = big_pool.tile([128, 2 * G_KD * KH], BF16, name="kTr2")  # 32KB/p
    Wv_sb = big_pool.tile([128, C_D * VD], BF16, name="Wv_sb")    # 16KB/p
    pT = big_pool.tile([128, KT * QBLK], BF16, name="pT")         # 16KB/p
    # V'sb layout: [128, kt(16) x 1024] (kt tile = k rows, cols = vd)
    Vp = big_pool.tile([128, KT * VD], BF16, name="Vp")           # 32KB/p
    ost_all = big_pool.tile([128, 2 * 1024], F32, name="ost_all")  # 8KB/p
    ostage = [ost_all[:, i * 1024:(i + 1) * 1024] for i in range(2)]

    # DRAM bounce buffers for the pair AllGathers (one pair per half so
    # each gather dispatches as soon as its half of the projection is done)
    dram_pool = tc.alloc_tile_pool(name="dramp", bufs=1, space="DRAM")
    kAG_in = [dram_pool.tile([128, 4 * 1024], BF16, name=f"kAG_in{i}")
              for i in range(2)]
    kAG_out = [dram_pool.tile([256, 4 * 1024], BF16, name=f"kAG_out{i}")
               for i in range(2)]
    vAG_in = [dram_pool.tile([128, 4 * VD], BF16, name=f"vAG_in{i}")
              for i in range(2)]
    vAG_out = [dram_pool.tile([256, 4 * VD], BF16, name=f"vAG_out{i}")
               for i in range(2)]

    # ---- projection-phase transients -----------------------------------
    proj_pool = tc.alloc_tile_pool(name="proj", bufs=1)
    Wk_sb = proj_pool.tile([128, C_D * KD], BF16, name="Wk_sb")   # 16KB/p
    Wq_sb = proj_pool.tile([128, C_D * KD], BF16, name="Wq_sb")   # 16KB/p
    stg = proj_pool.tile([128, G_KD * 1024], BF16, name="stg")    # 16KB/p

    def xin_tile(engine, src_ap, name):
        t = proj_pool.tile([128, C_D * 512], BF16, name=name, tag="xin",
                           bufs=3)
        engine.dma_start(out=t[:].rearrange("p (c x) -> p c x", c=C_D),
                         in_=src_ap.rearrange("(c p) x -> p c x", c=C_D))
        return t

    # scalar queue: kTo blocks (kproj gate), then vTo blocks, then Wv
    kin = [xin_tile(nc.scalar, kTo[:, b * 512:(b + 1) * 512], f"kin{b}")
           for b in range(2)]
    vin = [xin_tile(nc.scalar, vTo[:, b * 512:(b + 1) * 512], f"vin{b}")
           for b in range(2)]
    # Wv needed by V'o (~45us in); scalar queue, after the kproj gates.
    # gpsimd stays reserved for the bounce writes + collectives.
    nc.scalar.dma_start(out=Wv_sb[:].rearrange("p (c x) -> p c x", c=C_D),
                        in_=_chunked(Wv16, 0, C_D, VD))
    # sync queue: Wk first (kproj gate), then Wq/qT (consumed later, load
    # during kproj). Gather read-backs ride the scalar queue, which drains
    # its loads by ~25us -- keeping them off sync avoids blocking Wq/qT
    # behind the collectives (in-order FIFO per engine).
    nc.sync.dma_start(out=Wk_sb[:].rearrange("p (c x) -> p c x", c=C_D),
                      in_=_chunked(Wk, 0, C_D, KD))
    nc.sync.dma_start(out=Wq_sb[:].rearrange("p (c x) -> p c x", c=C_D),
                      in_=_chunked(Wq, 0, C_D, KD))
    qin = [xin_tile(nc.sync, qT[:, b * 512:(b + 1) * 512], f"qin{b}")
           for b in range(2)]

    psPro = tc.alloc_tile_pool(name="psPro", bufs=1, space="PSUM")

    # ---- k projection (own half): stg[g,1024] = (Wk^T kTo) + bk --------
    # g-half ordered so each half of stg completes early and its AllGather
    # dispatches while the other half is still computing.
    HG = G_KD // 2
    for gh in range(2):
        for blk in range(2):
            pps = [psPro.tile([128, 512], F32, name=f"kp{gh}_{blk}_{j}",
                              tag="pp", bufs=8) for j in range(HG)]
            for c in range(C_D):
                for j in range(HG):
                    g = gh * HG + j
                    nc.tensor.matmul(
                        pps[j][:],
                        Wk_sb[:, c * KD + g * 128:c * KD + (g + 1) * 128],
                        kin[blk][:, c * 512:(c + 1) * 512],
                        start=(c == 0), stop=(c == C_D - 1))
            for j in range(HG):
                g = gh * HG + j
                nc.vector.tensor_scalar(
                    out=stg[:, g * 1024 + blk * 512:
                            g * 1024 + (blk + 1) * 512],
                    in0=pps[j][:], scalar1=bkT[:, g:g + 1], scalar2=None,
                    op0=add)
        # ship this g-half, gather it, read both ranks' halves back
        h0 = gh * HG * 1024
        nc.gpsimd.dma_start(out=kAG_in[gh][:], in_=stg[:, h0:h0 + HG * 1024])
        nc.gpsimd.collective_compute(
            "AllGather", mybir.AluOpType.bypass,
            replica_groups=REPLICA_GROUPS,
            ins=[kAG_in[gh][:]], outs=[kAG_out[gh][:]])
        for r in range(2):
            nc.scalar.dma_start(
                out=kTr2[:, r * G_KD * KH + h0:r * G_KD * KH + h0 + HG * 1024],
                in_=kAG_out[gh][r * 128:(r + 1) * 128, :])

    # bv broadcast to all partitions via K=1 fp32 matmul
    for n in range(VD // 512):
        bc_ps = psPro.tile([128, 512], F32, name="bc_ps", tag="pp", bufs=8)
        nc.tensor.matmul(bc_ps[:], onesrow_f,
                         bv_stage[:, n * 512:(n + 1) * 512],
                         start=True, stop=True)
        nc.vector.tensor_copy(bvb[:, n * 512:(n + 1) * 512], bc_ps[:])

    # ---- V' (own half): stg[rt,1024] = value_own @ Wv + bv --------------
    # lhsT = vTo chunks [d, k-row slice], rhs = Wv chunks [d, vd cols]
    for blk in range(2):
        pps = [psPro.tile([128, 512], F32, name=f"vp{blk}_{i}", tag="pp",
                          bufs=8) for i in range(G_KD)]
        for c in range(C_D):
            for i in range(G_KD):
                rt, col = divmod(i, 2)
                nc.tensor.matmul(
                    pps[i][:],
                    vin[blk][:, c * 512 + rt * 128:c * 512 + (rt + 1) * 128],
                    Wv_sb[:, c * VD + col * 512:c * VD + (col + 1) * 512],
                    start=(c == 0), stop=(c == C_D - 1))
        for i in range(G_KD):
            rt, col = divmod(i, 2)
            krow = blk * 4 + rt
            nc.vector.tensor_tensor(
                out=stg[:, krow * 1024 + col * 512:krow * 1024 + (col + 1) * 512],
                in0=pps[i][:], in1=bvb[:, col * 512:(col + 1) * 512], op=add)
        # ship this krow-half, gather, read both ranks' quarters back
        b0 = blk * 4 * 1024
        nc.gpsimd.dma_start(out=vAG_in[blk][:], in_=stg[:, b0:b0 + 4 * 1024])
        nc.gpsimd.collective_compute(
            "AllGather", mybir.AluOpType.bypass,
            replica_groups=REPLICA_GROUPS,
            ins=[vAG_in[blk][:]], outs=[vAG_out[blk][:]])
        for r in range(2):
            nc.scalar.dma_start(
                out=Vp[:, r * 8 * VD + b0:r * 8 * VD + b0 + 4 * VD],
                in_=vAG_out[blk][r * 128:(r + 1) * 128, :])

    # ---- q projection: qTr[kd, q] = (Wq^T qT) + bq ----------------------
    for blk in range(NQB):
        pps = [psPro.tile([128, 512], F32, name=f"qp{blk}_{g}", tag="pp",
                          bufs=8) for g in range(G_KD)]
        for c in range(C_D):
            for g in range(G_KD):
                nc.tensor.matmul(
                    pps[g][:],
                    Wq_sb[:, c * KD + g * 128:c * KD + (g + 1) * 128],
                    qin[blk][:, c * 512:(c + 1) * 512],
                    start=(c == 0), stop=(c == C_D - 1))
        for g in range(G_KD):
            nc.vector.tensor_scalar(
                out=qTr[:, g * QS + blk * 512:g * QS + (blk + 1) * 512],
                in0=pps[g][:], scalar1=bqT[:, g:g + 1], scalar2=None, op0=add)

    psPro.release()
    proj_pool.release()

    # ===== main attention loop ==========================================
    # PSUM: sT(2) + rs(1) + acc(4) = 7 banks.
    psM = tc.alloc_tile_pool(name="psM", bufs=1, space="PSUM")
    rs_ps = psM.tile([128, 2 * NQS], F32, name="rs_ps", tag="rs")

    def kslice(g, kt):
        h, kk = divmod(kt, 8)
        base = h * G_KD * KH + g * KH + kk * 128
        return kTr2[:, base:base + 128]

    for qb in range(NQB):
        q0 = qb * QBLK
        # ---- phase A: sT = kTr^T qTr -> exp -> pT ; rowsums on PE ------
        for kt in range(KT):
            sT = psM.tile([128, QBLK], F32, name=f"sT{qb}_{kt}", tag="sT",
                          bufs=2)
            for g in range(G_KD):
                nc.tensor.matmul(
                    sT[:], kslice(g, kt),
                    qTr[:, g * QS + q0:g * QS + q0 + QBLK],
                    start=(g == 0), stop=(g == G_KD - 1))
            nc.scalar.activation(pT[:, kt * QBLK:(kt + 1) * QBLK], sT[:],
                                 Exp, scale=float(BETA))
            for qs in range(NQS):
                nc.tensor.matmul(
                    rs_ps[:, 2 * qs:2 * qs + 2],
                    pT[:, kt * QBLK + qs * 128:kt * QBLK + (qs + 1) * 128],
                    onesb[:],
                    start=(kt == 0 and qs == 0),
                    stop=(kt == KT - 1 and qs == NQS - 1),
                    skip_group_check=True)
        rrec = rrec_all[:, qb * 2 * NQS:(qb + 1) * 2 * NQS]
        nc.vector.reciprocal(rrec, rs_ps[:])

        # ---- phase B': out = (P V') * rrec  (V' already includes bv) ----
        for qs in range(NQS):
            accs = [psM.tile([128, 512], F32, name=f"ob{qb}_{qs}_{p}",
                             tag="acc", bufs=4) for p in range(2)]
            for kt in range(KT):
                for p in range(2):
                    nc.tensor.matmul(
                        accs[p][:],
                        pT[:, kt * QBLK + qs * 128:kt * QBLK + (qs + 1) * 128],
                        Vp[:, kt * VD + p * 512:kt * VD + (p + 1) * 512],
                        start=(kt == 0), stop=(kt == KT - 1))
            ost = ostage[qs % 2]
            for p in range(2):
                nc.vector.tensor_scalar(
                    out=ost[:, p * 512:(p + 1) * 512], in0=accs[p][:],
                    scalar1=rrec[:, 2 * qs:2 * qs + 1], scalar2=None, op0=mult)
            nc.sync.dma_start(
                out=out[q0 + qs * 128:q0 + (qs + 1) * 128, :], in_=ost[:])

    psM.release()
    dram_pool.release()
    big_pool.release()
    const_pool.release()


_NC_CACHE = {}


def _get_nc():
    if "nc" not in _NC_CACHE:
        _NC_CACHE["nc"] = build_kernel()
    return _NC_CACHE["nc"]


def kernel(query, key, value, Wq, bq, Wk, bk, Wv, bv):
    query = np.asarray(query, dtype=np.float32)
    key = np.asarray(key, dtype=np.float32)
    value = np.asarray(value, dtype=np.float32)
    Wq = np.asarray(Wq, dtype=np.float32)
    Wk = np.asarray(Wk, dtype=np.float32)
    Wv = np.asarray(Wv, dtype=np.float32)
    bq = np.asarray(bq, dtype=np.float32)
    bk = np.asarray(bk, dtype=np.float32)
    bv = np.ascontiguousarray(np.asarray(bv, dtype=np.float32))

    nc = _get_nc()
    in_maps = make_in_maps(query, key, value, Wq, bq, Wk, bk, Wv, bv)
    res = run_bass_kernel_spmd(nc, in_maps, list(range(N_CORES)))
    outp = np.empty((B, S, VD), dtype=np.float32)
    for core in range(N_CORES):
        b, h = divmod(core, 2)
        outp[b, h * QS:(h + 1) * QS, :] = res.results[core]["out"]
    return outp


def make_in_maps(query, key, value, Wq, bq, Wk, bk, Wv, bv):
    bf16 = ml_dtypes.bfloat16
    Wq16 = Wq.astype(bf16)
    Wk16 = Wk.astype(bf16)
    Wv16 = Wv.astype(bf16)
    bqk = np.ascontiguousarray(
        np.concatenate([bq.reshape(8, 128).T, bk.reshape(8, 128).T], axis=1)
        .astype(np.float32))
    in_maps = []
    for core in range(N_CORES):
        b, h = divmod(core, 2)
        sl = slice(h * KH, (h + 1) * KH)
        in_maps.append({
            "qT": np.ascontiguousarray(query[b, h * QS:(h + 1) * QS, :].T
                                       .astype(bf16)),
            "kTo": np.ascontiguousarray(key[b, sl, :].T.astype(bf16)),
            "vTo": np.ascontiguousarray(value[b, sl, :].T.astype(bf16)),
            "Wq": Wq16, "Wk": Wk16, "Wv16": Wv16,
            "bqk": bqk, "bv": bv,
        })
    return in_maps
